# revision 1
# baseline (speedup 1.0000x reference)
"""Conv1d (B=32, C_in=C_out=256, W=4096, K=3, pad=1) on 8 Trainium2 cores.

Strategy: data-parallel over batch (4 per core). Per core the conv is a sum
of 6 accumulated matmuls per 512-position output chunk: contraction over
(tap u in 0..2, ci_chunk in 0..1) with lhsT = weight[ci_chunk, :, co_chunk,
u].T ([128 ci x 128 co]) and rhs = a padded-x slice [128 ci x 512]. fp16
inputs (same PE rate as bf16, 8x lower error), fp32 PSUM accumulation, bias
added during the PSUM->SBUF drain on DVE.

Layout/scheduling choices (measured on HW):
- x arrives as quarter tiles (separate tiles - Tile tracks SBUF deps per
  whole tile) on the ACT HWDGE ring, ci-interleaved for batch 0, so the
  first matmuls start right after the ~7us framework prologue and PE never
  stalls on input data.
- each PSUM bank accumulates one chunk's 6 matmuls, then DVE drains it
  with the bias add; 8 banks cycle so PE streams back-to-back.
- outputs staged per (b, co) and flushed per finished 1024-position
  quarter so the tail only waits on a 0.5MB store.
"""

import numpy as np

F16 = np.float16

B, C, W, K = 32, 256, 4096, 3
NCORES = 8
BPC = B // NCORES          # batches per core
P = 128                    # partitions
CIC = C // P               # ci chunks
COC = C // P               # co chunks
NCH = 512                  # positions per matmul (one PSUM bank of fp32)
NCHUNKS = W // NCH         # position chunks per batch row
NQ = 4                     # x quarter tiles (batch 0)
QW = W // NQ               # 1024 positions per quarter

_cache = {}


def _build_program():
    import concourse.bass as bass
    import concourse.bacc as bacc
    import concourse.mybir as mybir
    from concourse import tile

    nc = bacc.Bacc(None, target_bir_lowering=False)
    # x, padded by one position on each side, pre-split in quarters with a
    # 2-column overlap: xq[b, ci, q] covers padded columns q*QW .. q*QW+QW+1.
    xq_d = nc.dram_tensor("xq", [BPC, CIC, NQ, P, QW + 2], mybir.dt.float16,
                          kind="ExternalInput")
    w_d = nc.dram_tensor("wt", [P, K * CIC * COC, P], mybir.dt.float16,
                         kind="ExternalInput")
    b_d = nc.dram_tensor("bb", [P, COC], mybir.dt.float32,
                         kind="ExternalInput")
    out_d = nc.dram_tensor("out", [BPC, COC, P, W], mybir.dt.float32,
                           kind="ExternalOutput")

    with tile.TileContext(nc) as tc:
        with (
            tc.tile_pool(name="wp", bufs=1) as wp,
            tc.tile_pool(name="xpool", bufs=BPC * CIC * NQ) as xpool,
            tc.tile_pool(name="opool", bufs=3) as opool,
            tc.tile_pool(name="pspool", bufs=8, space=bass.MemorySpace.PSUM) as pspool,
        ):
            w_sb = wp.tile([P, K * CIC * COC, P], mybir.dt.float16)
            nc.sync.dma_start(w_sb[:], w_d[:])
            b_sb = wp.tile([P, COC], mybir.dt.float32)
            nc.sync.dma_start(b_sb[:], b_d[:])

            # x quarter tiles; batch 0 first (quarter by quarter, ci
            # interleaved), then batches 1-3 with one DMA per (b, ci, q).
            x_sb = {}
            for b in range(BPC):
                for ci in range(CIC):
                    for q in range(NQ):
                        x_sb[(b, ci, q)] = xpool.tile(
                            [P, QW + 2], mybir.dt.float16,
                            name=f"xt_{b}_{ci}_{q}", tag="xt")
            for q in range(NQ):
                for ci in range(CIC):
                    nc.scalar.dma_start(x_sb[(0, ci, q)][:], xq_d[0, ci, q])
            for b in range(1, BPC):
                for ci in range(CIC):
                    for q in range(NQ):
                        nc.scalar.dma_start(x_sb[(b, ci, q)][:], xq_d[b, ci, q])

            def rhs(b, ci, n, u):
                # positions n*NCH .. n*NCH+511, tap offset u -> padded
                # columns n*NCH+u .. ; quarter q holds padded cols
                # q*QW .. q*QW+QW+1 at local offset -q*QW.
                q = (n * NCH) // QW
                lo = n * NCH + u - q * QW
                return x_sb[(b, ci, q)][:, lo:lo + NCH]

            NACC = K * CIC
            for b in range(BPC):
                for co in range(COC):
                    o_sb = opool.tile([P, W], mybir.dt.float32)
                    for n in range(NCHUNKS):
                        ps = pspool.tile([P, NCH], mybir.dt.float32,
                                         name=f"ps_{b}_{co}_{n}", tag="ps")
                        for k, (u, ci) in enumerate(
                                (u, ci) for u in range(K) for ci in range(CIC)):
                            nc.tensor.matmul(
                                ps[:], w_sb[:, (u * CIC + ci) * COC + co, :],
                                rhs(b, ci, n, u),
                                start=(k == 0), stop=(k == NACC - 1),
                            )
                        nc.vector.tensor_scalar_add(
                            o_sb[:, n * NCH:(n + 1) * NCH], ps[:],
                            b_sb[:, co:co + 1],
                        )
                        if n % 2 == 1:  # flush each finished quarter
                            qq = n // 2
                            nc.sync.dma_start(
                                out_d[b, co, :, qq * QW:(qq + 1) * QW],
                                o_sb[:, qq * QW:(qq + 1) * QW])
    nc.compile()
    return nc


def _prep_inputs(x, weight, bias):
    # x: [32,256,4096] f32 -> padded fp16 quarters [B, CIC, NQ, 128, QW+2]
    xp = np.zeros((B, CIC, P, W + 2), F16)
    xp[:, :, :, 1:W + 1] = x.reshape(B, CIC, P, W).astype(F16)
    xq = np.empty((B, CIC, NQ, P, QW + 2), F16)
    for q in range(NQ):
        xq[:, :, q] = xp[:, :, :, q * QW:q * QW + QW + 2]
    # weight: [co, ci, u] -> [ci_in, (u, ci_c, co_c), co_in]
    wt = weight.reshape(COC, P, CIC, P, K)          # [co_c, co_in, ci_c, ci_in, u]
    w_host = np.ascontiguousarray(
        wt.transpose(3, 4, 2, 0, 1)                 # [ci_in, u, ci_c, co_c, co_in]
    ).reshape(P, K * CIC * COC, P).astype(F16)
    b_host = np.ascontiguousarray(bias.reshape(COC, P).T).astype(np.float32)
    return xq, w_host, b_host


def run(x, weight, bias, trace=False):
    from concourse.bass_utils import run_bass_kernel_spmd

    if "nc" not in _cache:
        _cache["nc"] = _build_program()
    nc = _cache["nc"]

    xq, w_host, b_host = _prep_inputs(
        np.asarray(x, np.float32), np.asarray(weight, np.float32),
        np.asarray(bias, np.float32))
    in_maps = [
        {"xq": xq[c * BPC:(c + 1) * BPC], "wt": w_host, "bb": b_host}
        for c in range(NCORES)
    ]
    res = run_bass_kernel_spmd(nc, in_maps, list(range(NCORES)), trace=trace)
    out = np.concatenate(
        [res.results[c]["out"].reshape(BPC, C, W) for c in range(NCORES)], axis=0)
    return out, res


def kernel(x, weight, bias):
    out, _ = run(x, weight, bias, trace=False)
    return out



# revision 2
# speedup vs baseline: 1.0088x; 1.0088x over previous
"""Conv1d (B=32, C_in=C_out=256, W=4096, K=3, pad=1) on 8 Trainium2 cores, v4.

Data-parallel over batch (4 per core); per core the conv is 6 accumulated
matmuls per 512-position PSUM chunk (taps x ci-chunks), fp32 PSUM, bias
added during the DVE drain.

Key measured facts driving the design:
- exec_time window starts at a fixed ~5.7us and ends after a ~7.3us
  framework teardown (57 serial semaphore waits per engine) that does NOT
  shrink with DMA count. The levers are: PE start time, PE stream density,
  and last-DMA-complete time.
- Both HWDGE rings (SP + ACT) share one ~300-380GB/s pool; the early window
  is delivery-bound, so x is loaded as fp8-e3m4 (half the bytes, exact
  upconvert in the PE; rhs fp8e3 x lhsT fp16 runs at the same 1 cycle/row)
  and the latency-critical stream goes first on both rings, bulk after.
- Outputs staged+stored as fp16 (host upcasts); last (b,co) flushed in
  pieces on the by-then-idle ACT ring so the tail is one small store.
- Dummy matmuls on a memset tile bridge the PE from its preamble to the
  first real matmul so the DVFS ramp overlaps the DMA wait.
"""

import numpy as np
import ml_dtypes

F16 = np.float16
F8 = ml_dtypes.float8_e3m4

B, C, W, K = 32, 256, 4096, 3
NCORES = 8
BPC = B // NCORES          # batches per core
P = 128                    # partitions
CIC = C // P               # ci chunks
COC = C // P               # co chunks
NCH = 512                  # positions per matmul (one fp32 PSUM bank)
NCHUNKS = W // NCH
WP = W + 2                 # padded row

# b0 x pieces (padded column ranges, 2-col overlap at boundaries)
PIECES = [(0, 514), (512, 2050), (2048, 4098)]
# (b3,co1) output flush boundaries: flush after chunk n covering [lo, hi)
LAST_FLUSH = {1: (0, 1024), 3: (1024, 2048), 5: (2048, 3072),
              6: (3072, 3584)}
# final chunk split across both rings so the tail is ~0.35us
LAST_SPLIT = [(3584, 3840), (3840, 4096)]
WU_W = [512] * 3 + [256] * 2   # warm-up matmul widths

_cache = {}


def _build_program():
    import concourse.bass as bass
    import concourse.bacc as bacc
    import concourse.mybir as mybir
    from concourse import tile

    nc = bacc.Bacc(None, target_bir_lowering=False)
    # x: [b][p][ci][padded w] fp8-e3m4; one DMA per batch covers both ci
    x_d = nc.dram_tensor("xq", [BPC, P, CIC, WP], mybir.dt.float8e3,
                         kind="ExternalInput")
    # weights: 12 [ci_in, co_in] blocks ordered (co, ci, u)
    w_d = nc.dram_tensor("wt", [P, COC * CIC * K, P], mybir.dt.float16,
                         kind="ExternalInput")
    b_d = nc.dram_tensor("bb", [P, COC], mybir.dt.float32,
                         kind="ExternalInput")
    out_d = nc.dram_tensor("out", [BPC, COC, P, W], mybir.dt.float16,
                           kind="ExternalOutput")

    with tile.TileContext(nc) as tc:
        with (
            tc.tile_pool(name="wp", bufs=1) as wp,
            tc.tile_pool(name="xpool", bufs=1) as xpool,
            tc.tile_pool(name="opool", bufs=4) as opool,
            tc.tile_pool(name="pspool", bufs=5, space=bass.MemorySpace.PSUM) as pspool,
            tc.tile_pool(name="wups", bufs=2, space=bass.MemorySpace.PSUM) as wups,
        ):
            # ---- PE warm-up: memset a scratch tile, matmul on it until
            # real data arrives (never read back).
            # gpsimd's preamble ends ~2us before DVE's, so the warm-up
            # can start that much earlier
            wu = wp.tile([P, 640], mybir.dt.float16, name="wu", tag="wu")
            nc.gpsimd.memset(wu[:], 0.0)
            for i, wuw in enumerate(WU_W):
                psw = wups.tile([P, 512], mybir.dt.float32,
                                name=f"psw_{i}", tag="psw")
                nc.tensor.matmul(psw[:, 0:wuw], wu[:, 0:128],
                                 wu[:, 128:128 + wuw], start=True, stop=True)

            # ---- x tiles: b0 per (ci, piece); b1-3 one tile per batch
            x0 = {}
            for ci in range(CIC):
                for pi, (s, e) in enumerate(PIECES):
                    x0[(ci, pi)] = xpool.tile(
                        [P, e - s], mybir.dt.float8e3,
                        name=f"x0_{ci}_{pi}", tag=f"x0_{ci}_{pi}")
            xb = {}
            for b in range(1, BPC):
                xb[b] = xpool.tile([P, CIC, WP], mybir.dt.float8e3,
                                   name=f"xb_{b}", tag=f"xb_{b}")

            # ---- weight/bias tiles: w_a/w_b = (co0, ci0/ci1) taps,
            # w_c = all of co1 (needed ~10us later)
            w_a = wp.tile([P, K, P], mybir.dt.float16, name="w_a", tag="w_a")
            w_b = wp.tile([P, K, P], mybir.dt.float16, name="w_b", tag="w_b")
            w_c = wp.tile([P, K * CIC, P], mybir.dt.float16, name="w_c", tag="w_c")
            b_sb = wp.tile([P, COC], mybir.dt.float32, name="bb", tag="bb")

            # ---- DMA issue order: latency-critical first on BOTH rings
            # (they share bandwidth), bulk strictly after.
            # ACT: all b0 pieces ci-interleaved in consumption order, then
            # the bulk b1-3 batches (+ final flushes, issued from the loop).
            # SP: only the weights/bias early (done by ~10us), then idle
            # until the per-(b,co) stores start -- so ACT's critical
            # stream gets the whole pool during the PE start window.
            for pi in range(len(PIECES)):
                s, e = PIECES[pi]
                for ci in range(CIC):
                    nc.scalar.dma_start(x0[(ci, pi)][:], x_d[0, :, ci, s:e])
            nc.sync.dma_start(w_a[:], w_d[:, 0:K])
            nc.sync.dma_start(w_b[:], w_d[:, K:2 * K])
            nc.sync.dma_start(b_sb[:], b_d[:])
            nc.sync.dma_start(w_c[:], w_d[:, 2 * K:4 * K])
            for b in range(1, BPC):
                nc.scalar.dma_start(xb[b][:], x_d[b])

            def wslice(co, ci, u):
                if co == 0:
                    return (w_a if ci == 0 else w_b)[:, u, :]
                return w_c[:, ci * K + u, :]

            def rhs(b, ci, n, u):
                off = n * NCH + u
                if b == 0:
                    for pi, (s, e) in enumerate(PIECES):
                        if s <= off and off + NCH <= e:
                            return x0[(ci, pi)][:, off - s:off - s + NCH]
                    raise AssertionError((n, u))
                return xb[b][:, ci, off:off + NCH]

            NACC = K * CIC
            last = (BPC - 1, COC - 1)
            for b in range(BPC):
                for co in range(COC):
                    o_sb = opool.tile([P, W], mybir.dt.float16)
                    for n in range(NCHUNKS):
                        ps = pspool.tile([P, NCH], mybir.dt.float32,
                                         name=f"ps_{b}_{co}_{n}", tag="ps")
                        k = 0
                        for ci in range(CIC):
                            for u in range(K):
                                nc.tensor.matmul(
                                    ps[:], wslice(co, ci, u), rhs(b, ci, n, u),
                                    start=(k == 0), stop=(k == NACC - 1))
                                k += 1
                        nc.vector.tensor_scalar_add(
                            o_sb[:, n * NCH:(n + 1) * NCH], ps[:],
                            b_sb[:, co:co + 1])
                        if (b, co) == last and n in LAST_FLUSH:
                            lo, hi = LAST_FLUSH[n]
                            nc.scalar.dma_start(out_d[b, co, :, lo:hi],
                                                o_sb[:, lo:hi])
                        if (b, co) == last and n == NCHUNKS - 1:
                            # final chunk split across both (idle) rings
                            (l0, h0), (l1, h1) = LAST_SPLIT
                            nc.sync.dma_start(out_d[b, co, :, l0:h0],
                                              o_sb[:, l0:h0])
                            nc.scalar.dma_start(out_d[b, co, :, l1:h1],
                                                o_sb[:, l1:h1])
                    if (b, co) != last:
                        nc.sync.dma_start(out_d[b, co], o_sb[:])
    nc.compile()
    return nc


def _prep_inputs(x, weight, bias):
    # x: [32,256,4096] f32 -> padded fp8-e3m4 [B, P, CIC, W+2]
    xp = np.zeros((B, P, CIC, WP), F8)
    xp[:, :, :, 1:W + 1] = x.reshape(B, CIC, P, W).transpose(0, 2, 1, 3).astype(F8)
    # weight: [co, ci, u] -> [ci_in, (co_c, ci_c, u), co_in]
    wt = weight.reshape(COC, P, CIC, P, K)          # [co_c, co_in, ci_c, ci_in, u]
    w_host = np.ascontiguousarray(
        wt.transpose(3, 0, 2, 4, 1)                 # [ci_in, co_c, ci_c, u, co_in]
    ).reshape(P, COC * CIC * K, P).astype(F16)
    b_host = np.ascontiguousarray(bias.reshape(COC, P).T).astype(np.float32)
    return xp, w_host, b_host


def run(x, weight, bias, trace=False):
    from concourse.bass_utils import run_bass_kernel_spmd

    if "nc" not in _cache:
        _cache["nc"] = _build_program()
    nc = _cache["nc"]

    xp, w_host, b_host = _prep_inputs(
        np.asarray(x, np.float32), np.asarray(weight, np.float32),
        np.asarray(bias, np.float32))
    in_maps = [
        {"xq": xp[c * BPC:(c + 1) * BPC], "wt": w_host, "bb": b_host}
        for c in range(NCORES)
    ]
    res = run_bass_kernel_spmd(nc, in_maps, list(range(NCORES)), trace=trace)
    out = np.concatenate(
        [res.results[c]["out"].reshape(BPC, C, W) for c in range(NCORES)],
        axis=0).astype(np.float32)
    return out, res


def kernel(x, weight, bias):
    out, _ = run(x, weight, bias, trace=False)
    return out
